# revision 20
# baseline (speedup 1.0000x reference)
"""Trainium2 Bass kernel for nn_AspectModel (span-attention aspect tagger).

Strategy: batch-shard the 32 sentences 4-per-core across 8 NeuronCores; route
each fragment (host-side) to the core owning its sentence, padded to 64 slots
per sentence (256 slots/core).  All heavy math runs on-chip:
  - span features (l_word / word_state / r_word) via a masks-matmul against
    the sentence hidden states (one-hot + in-span masks built on-chip),
  - v = span @ att_w and c = span @ att_b as dense matmuls over all slots,
  - attention scores via a PE matmul of V against the transposed memory
    (transpose done by the DMA xbar in bf16),
  - masked softmax (fused exp+sum) and mix via a second masks-matmul,
  - tag logits + log_softmax.
Matmul operands are cast to bf16 on-chip (f32 PSUM accumulation); the f32
tensor-engine path costs 2 passes per matmul, bf16 costs 1.
Each core returns its own [256, 5] slot outputs; the host scatters them back
into the full [1024, 5] output.  No collectives needed.
"""

import sys
import types

import numpy as np

# Optional shim so run_bass_kernel_spmd(trace=True) works in containers where
# antenv.axon_hooks is missing (profiling only; correctness path unaffected).
try:
    import antenv.axon_hooks  # noqa: F401
except ImportError:
    try:
        from trn_agent_boot.trn_boot import _ntff_profile_via_ctypes

        _hook = _ntff_profile_via_ctypes("/opt/axon/libaxon_pjrt.so")
        _mod = types.ModuleType("antenv.axon_hooks")
        _mod.get_axon_ntff_profile_hook = lambda: _hook
        _mod.set_axon_ntff_profile_hook = lambda h: None
        sys.modules["antenv.axon_hooks"] = _mod
    except Exception:
        pass

import concourse.bass as bass  # noqa: E402
import concourse.tile as tile  # noqa: E402
from concourse import bacc, mybir  # noqa: E402
from concourse import bass_utils  # noqa: E402
from concourse.bass_utils import run_bass_kernel_spmd  # noqa: E402

# No artifact bucket in the sandbox; make tracing's upload step a no-op.
bass_utils.upload_artifacts = lambda tmpdir: f"local:{tmpdir}"

F32 = mybir.dt.float32
BF16 = mybir.dt.bfloat16
I32 = mybir.dt.int32
ALU = mybir.AluOpType
ACT = mybir.ActivationFunctionType

B, S, D, F, T = 32, 256, 512, 1024, 5
NCORES = 8
SEN = 4          # sentences per core
G = 64           # fragment slots per sentence
C = SEN * G      # 256 fragment slots per core
D3 = 3 * D

TRACE = False
LAST_RESULT = None  # BassKernelResults of the most recent run (for test.py)

_compiled = {}


def _build(seq_len: float):
    """Build + compile the per-core SPMD graph (identical on all 8 cores)."""
    nc = bacc.Bacc("TRN2", target_bir_lowering=False, debug=False,
                   num_devices=NCORES)

    # All inputs are laid out host-side as [partition, free] so every DMA is a
    # dense per-partition contiguous read.
    x_d = nc.dram_tensor("x", [128, 2 * SEN, D], F32, kind="ExternalInput")
    aw_d = nc.dram_tensor("aw", [128, 12, D], F32, kind="ExternalInput")
    ab_d = nc.dram_tensor("ab", [128, 12], F32, kind="ExternalInput")
    tw_d = nc.dram_tensor("tw", [128, 16, T], F32, kind="ExternalInput")
    tb_d = nc.dram_tensor("tb", [1, T], F32, kind="ExternalInput")
    fs_r_d = nc.dram_tensor("fs_r", [1, C], F32, kind="ExternalInput")
    fm_r_d = nc.dram_tensor("fm_r", [1, C], F32, kind="ExternalInput")
    fs_c_d = nc.dram_tensor("fs_c", [128, 2], F32, kind="ExternalInput")
    fm_c_d = nc.dram_tensor("fm_c", [128, 2], F32, kind="ExternalInput")
    ln_c_d = nc.dram_tensor("ln_c", [128, 2], F32, kind="ExternalInput")
    out_d = nc.dram_tensor("out", [2, 128, T], F32, kind="ExternalOutput")

    with tile.TileContext(nc) as tc:
        with (
            tc.tile_pool(name="persist", bufs=1) as pp,
            tc.tile_pool(name="work", bufs=2) as wp,
            tc.tile_pool(name="psum", bufs=2, space="PSUM") as psp,
        ):
            # ---- persistent SBUF tensors ----
            x_sb = pp.tile([128, 2 * SEN, D], F32, tag="x_sb")
            x_bf = pp.tile([128, 2 * SEN, D], BF16, tag="x_bf")
            aw_sb = pp.tile([128, 12, D], F32, tag="aw_sb")
            aw_bf = pp.tile([128, 12, D], BF16, tag="aw_bf")
            ab_sb = pp.tile([128, 12], F32, tag="ab_sb")
            ab_bf = pp.tile([128, 12], BF16, tag="ab_bf")
            tw_sb = pp.tile([128, 16, T], F32, tag="tw_sb")
            tw_bf = pp.tile([128, 16, T], BF16, tag="tw_bf")
            tb_sb = pp.tile([1, T], F32, tag="tb_sb")
            tbb = pp.tile([128, T], F32, tag="tbb")
            fs_r = pp.tile([1, C], F32, tag="fs_r")
            fm_r = pp.tile([1, C], F32, tag="fm_r")
            fs_c = pp.tile([128, 2], F32, tag="fs_c")
            fm_c = pp.tile([128, 2], F32, tag="fm_c")
            ln_c = pp.tile([128, 2], F32, tag="ln_c")
            iota_i = pp.tile([128, S], I32, tag="iota_i")
            iota_f = pp.tile([128, S], F32, tag="iota_f")
            iota_n = pp.tile([128, S], F32, tag="iota_n")
            fs_b = pp.tile([128, C], F32, tag="fs_b")
            fm_b = pp.tile([128, C], F32, tag="fm_b")
            mkT = [pp.tile([128, 3, C], BF16, tag=f"mkT{k}", name=f"mkT{k}")
                   for k in range(2)]
            spanT = pp.tile([128, 12, C], BF16, tag="spanT")
            v_sb = pp.tile([128, 4, C], BF16, tag="v_sb")
            c_sb = pp.tile([128, 2], F32, tag="c_sb")
            memT = pp.tile([128, SEN, 2, 4, 128], BF16, tag="memT")
            mixT = pp.tile([128, 4, C], BF16, tag="mixT")

            # ---- input DMAs.  The sync HWDGE ring carries only x and aw (the
            # two big loads, x first); everything small plus the SBUF->SBUF
            # transposes go on the scalar HWDGE ring so they never queue
            # behind the big transfers.
            nc.sync.dma_start(fs_r[:], fs_r_d.ap())
            nc.sync.dma_start(fm_r[:], fm_r_d.ap())
            nc.sync.dma_start(fs_c[:], fs_c_d.ap())
            nc.sync.dma_start(fm_c[:], fm_c_d.ap())
            nc.sync.dma_start(ln_c[:], ln_c_d.ap())
            nc.scalar.dma_start(tb_sb[:], tb_d.ap())
            nc.scalar.dma_start(ab_sb[:], ab_d.ap())
            nc.scalar.dma_start(tw_sb[:], tw_d.ap())
            for half in range(2):
                nc.sync.dma_start(x_sb[:, half * 4:(half + 1) * 4, :],
                                  x_d.ap()[:, half * 4:(half + 1) * 4, :])
            for third in range(3):
                nc.sync.dma_start(aw_sb[:, third * 4:(third + 1) * 4, :],
                                  aw_d.ap()[:, third * 4:(third + 1) * 4, :])

            # ---- constants + masksT first (keeps the DVE queue from
            # stalling on the x transfer behind the cast ops) ----
            neg4 = pp.tile([128, 1], F32, tag="neg4")
            nc.gpsimd.memset(neg4[:], -1.0e4)
            nc.gpsimd.iota(iota_i[:], pattern=[[1, S]], channel_multiplier=0)
            nc.vector.tensor_copy(iota_f[:], iota_i[:])
            nc.vector.tensor_scalar_mul(iota_n[:], iota_f[:], -1.0)
            nc.gpsimd.partition_broadcast(fs_b[:], fs_r[:])
            nc.gpsimd.partition_broadcast(fm_b[:], fm_r[:])
            nc.gpsimd.partition_broadcast(tbb[:], tb_sb[:])

            # masksT in [s, slot] layout (rhs of the span masks-matmul):
            # component 0: one-hot at frag_s, 1: in-span, 2: one-hot at fe-1.
            # partition-index iota per s-chunk: value = k*128 + p
            piota_i = pp.tile([128, 2], I32, tag="piota_i")
            piota = pp.tile([128, 2], F32, tag="piota")
            nc.gpsimd.iota(piota_i[:], pattern=[[128, 2]], channel_multiplier=1)
            nc.vector.tensor_copy(piota[:], piota_i[:])

            for k in range(2):
                ip = piota[:, k:k + 1]  # [128,1] = s index in chunk k
                ge = wp.tile([128, C], F32, tag="msk_ge")
                le = wp.tile([128, C], F32, tag="msk_le")
                # fs <= s  and  fe-1 >= s
                nc.vector.tensor_scalar(ge[:], fs_b[:], ip, None, op0=ALU.is_le)
                nc.vector.tensor_scalar(le[:], fm_b[:], ip, None, op0=ALU.is_ge)
                nc.vector.tensor_scalar(mkT[k][:, 0, :], fs_b[:], ip, None,
                                        op0=ALU.is_equal)
                nc.vector.tensor_tensor(mkT[k][:, 1, :], ge[:], le[:],
                                        op=ALU.mult)
                nc.vector.tensor_scalar(mkT[k][:, 2, :], fm_b[:], ip, None,
                                        op0=ALU.is_equal)

            # ---- bf16 casts of matmul operands ----
            for c8 in range(2 * SEN):
                nc.vector.tensor_copy(x_bf[:, c8, :], x_sb[:, c8, :])
            for kk in range(12):
                nc.any.tensor_copy(aw_bf[:, kk, :], aw_sb[:, kk, :])
            nc.any.tensor_copy(ab_bf[:], ab_sb[:])
            nc.any.tensor_copy(tw_bf[:], tw_sb[:])

            # ---- memT[d, s] via DMA xbar transpose (bf16) ----
            # One big transpose per 2-sentence half: [128, 2048] -> logical
            # [2048, 128]; row r = (l*2+k)*512 + d lands at chunk
            # m = (l*2+k)*4 + dj, partition d%128.  memT is [128, l, k, dj, s'].
            for hf in range(2):
                nc.scalar.dma_start_transpose(
                    memT[:, 2 * hf:2 * hf + 2, :, :, :],
                    x_bf[:, hf * 4:(hf + 1) * 4, :])

            # ---- span masks-matmul: spanT[3D, C] (l_word | word_state | r_word)
            for l in range(SEN):
                for j0 in range(2):  # pairs of D chunks of the word vectors
                    ps = psp.tile([128, 2, 3, G], F32, tag="psm")
                    for dj in range(2):
                        j = j0 * 2 + dj
                        for k in range(2):
                            nc.tensor.matmul(
                                ps[:, dj, :, :],
                                x_bf[:, l * 2 + k, j * 128:(j + 1) * 128],
                                mkT[k][:, :, l * G:(l + 1) * G],
                                start=(k == 0), stop=(k == 1),
                            )
                    nc.any.tensor_copy(
                        spanT[:, j0 * 6:j0 * 6 + 6, l * G:(l + 1) * G], ps[:])

            # ---- v = span @ att_w  (stored transposed: V[d, slot])
            for m0 in range(2):
                pv = psp.tile([128, 2, C], F32, tag="p2k")
                for mj in range(2):
                    m = m0 * 2 + mj
                    for kk in range(12):
                        nc.tensor.matmul(
                            pv[:, mj, :],
                            aw_bf[:, kk, m * 128:(m + 1) * 128],
                            spanT[:, kk, :],
                            start=(kk == 0), stop=(kk == 11),
                        )
                nc.any.tensor_copy(v_sb[:, m0 * 2:m0 * 2 + 2, :], pv[:])

            # ---- c = span @ att_b  ([slot, 1] per 128-slot chunk)
            for q in range(2):
                pc = psp.tile([128, 1], F32, tag="pout")
                for kk in range(12):
                    nc.tensor.matmul(
                        pc[:],
                        spanT[:, kk, q * 128:(q + 1) * 128],
                        ab_bf[:, kk:kk + 1],
                        start=(kk == 0), stop=(kk == 11),
                    )
                nc.any.tensor_copy(c_sb[:, q:q + 1], pc[:])

            # ---- per sentence-pair attention + mix + logits ----
            sh_t = pp.tile([128, 2, T], F32, tag="sh_t")
            se_t = pp.tile([128, 2], F32, tag="se_t")
            ex2_insts = []
            for q in range(2):
                gp = psp.tile([128, S], F32, tag="p2k")
                for h in range(2):
                    l = 2 * q + h
                    for dj in range(4):
                        nc.tensor.matmul(
                            gp[h * G:(h + 1) * G, :],
                            v_sb[:, dj, l * G:(l + 1) * G],
                            memT[:, l, :, dj, :],
                            start=(dj == 0), stop=(dj == 3),
                        )

                fs_q = fs_c[:, q:q + 1]
                fm_q = fm_c[:, q:q + 1]
                ln_q = ln_c[:, q:q + 1]
                c_q = c_sb[:, q:q + 1]

                t1 = wp.tile([128, S], F32, tag="t1")
                t2 = wp.tile([128, S], F32, tag="t2")
                dm = wp.tile([128, S], F32, tag="dm")
                pwr = wp.tile([128, S], F32, tag="pwr")
                noti = wp.tile([128, S], F32, tag="noti")
                klen = wp.tile([128, S], F32, tag="klen")
                kp = wp.tile([128, S], F32, tag="kp")
                pw = wp.tile([128, S], F32, tag="pw")
                sg = wp.tile([128, S], F32, tag="sg")
                th = wp.tile([128, S], F32, tag="th")
                thm = wp.tile([128, S], F32, tag="thm")
                ex = wp.tile([128, S], F32, tag="ex")
                wf = wp.tile([128, S], BF16, tag="wf")
                den = wp.tile([128, 1], F32, tag="den")
                rden = wp.tile([128, 1], F32, tag="rden")

                # pos-distance / pos-weight / masks (slot-major [slot, s])
                nc.vector.tensor_scalar(t1[:], iota_n[:], fs_q, None,
                                        op0=ALU.add)            # fs - s
                nc.vector.tensor_scalar(t2[:], iota_f[:], fm_q, None,
                                        op0=ALU.subtract)       # s - (fe-1)
                nc.vector.tensor_tensor(dm[:], t1[:], t2[:], op=ALU.max)
                nc.vector.tensor_scalar(pwr[:], dm[:], -1.0 / seq_len, 1.0,
                                        op0=ALU.mult, op1=ALU.add)
                nc.vector.tensor_single_scalar(noti[:], dm[:], 0.0,
                                               op=ALU.is_gt)    # not in span
                nc.vector.tensor_scalar(klen[:], iota_f[:], ln_q, None,
                                        op0=ALU.is_lt)          # s < len
                nc.vector.tensor_tensor(kp[:], noti[:], klen[:], op=ALU.mult)
                nc.vector.tensor_tensor(pw[:], pwr[:], noti[:], op=ALU.mult)

                # scores -> tanh -> masked exp (+denominator) -> mix weights
                nc.vector.tensor_tensor(sg[:], pw[:], gp[:], op=ALU.mult)
                nc.scalar.activation(th[:], sg[:], ACT.Tanh, bias=c_q)
                nc.vector.scalar_tensor_tensor(thm[:], th[:], 1.0e4, kp[:],
                                               op0=ALU.add, op1=ALU.mult)
                nc.scalar.activation(ex[:], thm[:], ACT.Exp, bias=neg4[:],
                                     accum_out=den[:])
                nc.vector.reciprocal(rden[:], den[:])
                nc.vector.scalar_tensor_tensor(wf[:], ex[:], rden[:], pw[:],
                                               op0=ALU.mult, op1=ALU.mult)

                # transpose mix weights to [s, slot] via DMA xbar (bf16)
                wT = wp.tile([128, 2, 128], BF16, tag="wT")
                eng = nc.sync if q == 0 else nc.scalar
                eng.dma_start_transpose(wT[:], wf[:])

                # mixT[d, slot] = sum_s mem[s, d] * w[slot, s]
                for h in range(2):
                    l = 2 * q + h
                    pm = psp.tile([128, 4, G], F32, tag="pmix")
                    for dj in range(4):
                        for k in range(2):
                            nc.tensor.matmul(
                                pm[:, dj, :],
                                x_bf[:, l * 2 + k, dj * 128:(dj + 1) * 128],
                                wT[:, k, h * G:(h + 1) * G],
                                start=(k == 0), stop=(k == 1),
                            )
                    nc.any.tensor_copy(mixT[:, :, l * G:(l + 1) * G], pm[:])

                # logits = [span | mix] @ tag_w.T + tag_b
                pl = psp.tile([128, T], F32, tag="pout")
                for kk in range(12):
                    nc.tensor.matmul(
                        pl[:], spanT[:, kk, q * 128:(q + 1) * 128],
                        tw_bf[:, kk, :], start=(kk == 0), stop=False)
                for dj in range(4):
                    nc.tensor.matmul(
                        pl[:], mixT[:, dj, q * 128:(q + 1) * 128],
                        tw_bf[:, 12 + dj, :], start=False, stop=(dj == 3))

                lg = wp.tile([128, T], F32, tag="lg")
                mx = wp.tile([128, 1], F32, tag="mx")
                nc.vector.tensor_tensor(lg[:], pl[:], tbb[:], op=ALU.add)
                nc.vector.tensor_reduce(mx[:], lg[:], axis=mybir.AxisListType.X,
                                        op=ALU.max)
                nc.vector.tensor_scalar(sh_t[:, q, :], lg[:], mx[:], None,
                                        op0=ALU.subtract)
                ex2 = wp.tile([128, T], F32, tag="ex2")
                ex2_insts.append(
                    nc.scalar.activation(ex2[:], sh_t[:, q, :], ACT.Exp,
                                         accum_out=se_t[:, q:q + 1]))

            # ---- log-softmax epilogue (Ln grouped after all Exp/Tanh) ----
            for q in range(2):
                lse = wp.tile([128, 1], F32, tag="lse")
                res = wp.tile([128, T], F32, tag="res")
                ln_inst = nc.scalar.activation(lse[:], se_t[:, q:q + 1],
                                               ACT.Ln)
                tile.add_dep_helper(ln_inst.ins, ex2_insts[1].ins, sync=False,
                                    reason="group Ln after all Exp")
                nc.vector.tensor_scalar(res[:], sh_t[:, q, :], lse[:], None,
                                        op0=ALU.subtract)
                nc.sync.dma_start(out_d.ap()[q], res[:])

    nc.compile()
    return nc


def _host_prep(en_output, lengths, frag_b, frag_s, frag_e, att_w, att_b,
               tag_w, tag_b):
    """Shard + relayout inputs.  Returns (in_maps, assign, overflow)."""
    # replicated weights, permuted so spanT chunk kk = 3*j + comp maps to
    # att rows comp*512 + j*128 : .. + 128.
    perm = np.concatenate([
        np.arange(comp * D + j * 128, comp * D + (j + 1) * 128)
        for j in range(4) for comp in range(3)
    ])
    aw_np = att_w[perm].reshape(12, 128, D).transpose(1, 0, 2).reshape(128, 12, D)
    ab_np = att_b[perm].reshape(12, 128).T.copy()
    tw_rows = np.concatenate([tag_w[:, perm].T,
                              tag_w[:, D3:].T], axis=0)  # [2048, 5]
    tw_np = tw_rows.reshape(16, 128, T).transpose(1, 0, 2).reshape(128, 16, T)
    tb_np = tag_b.reshape(1, T)

    aw_np = np.ascontiguousarray(aw_np, dtype=np.float32)
    ab_np = np.ascontiguousarray(ab_np, dtype=np.float32)
    tw_np = np.ascontiguousarray(tw_np, dtype=np.float32)
    tb_np = np.ascontiguousarray(tb_np, dtype=np.float32)

    assign = np.full((F, 2), -1, dtype=np.int64)  # (core, slot) per fragment
    counts = np.zeros((NCORES, SEN), dtype=np.int64)
    overflow = []
    in_maps = []

    fs_slot = np.zeros((NCORES, C), np.float32)
    fm_slot = np.zeros((NCORES, C), np.float32)
    ln_slot = np.full((NCORES, C), float(S), np.float32)

    for i in range(F):
        b = int(frag_b[i])
        core, l = b // SEN, b % SEN
        k = counts[core, l]
        if k >= G:
            overflow.append(i)
            continue
        counts[core, l] += 1
        slot = l * G + k
        assign[i] = (core, slot)
        fs_slot[core, slot] = frag_s[i]
        fm_slot[core, slot] = frag_e[i] - 1
        ln_slot[core, slot] = lengths[b]

    for core in range(NCORES):
        xs = en_output[core * SEN:(core + 1) * SEN]  # [4, 256, 512]
        x_np = np.ascontiguousarray(
            xs.reshape(SEN, 2, 128, D).transpose(2, 0, 1, 3)
              .reshape(128, 2 * SEN, D), dtype=np.float32)
        in_maps.append({
            "x": x_np, "aw": aw_np, "ab": ab_np, "tw": tw_np, "tb": tb_np,
            "fs_r": fs_slot[core].reshape(1, C),
            "fm_r": fm_slot[core].reshape(1, C),
            "fs_c": np.ascontiguousarray(fs_slot[core].reshape(2, 128).T),
            "fm_c": np.ascontiguousarray(fm_slot[core].reshape(2, 128).T),
            "ln_c": np.ascontiguousarray(ln_slot[core].reshape(2, 128).T),
        })
    return in_maps, assign, overflow


def _host_fragment(en_output, lengths, s, e, b, att_w, att_b, tag_w, tag_b,
                   seq_len):
    """Numpy fallback for (vanishingly rare) slot-overflow fragments."""
    mem = en_output[b].astype(np.float64)
    ws = mem[s:e].sum(0)
    span = np.concatenate([mem[s], ws, mem[e - 1]])
    pos = np.arange(S)
    in_span = (pos >= s) & (pos < e)
    att_mask = in_span | (pos >= lengths[b])
    dis = np.where(pos < s, s - pos,
                   np.where(pos >= e, pos - e + 1, seq_len)).astype(np.float64)
    pwv = 1.0 - dis / seq_len
    fin = pwv[:, None] * mem
    v = span @ att_w.astype(np.float64)
    c = span @ att_b.astype(np.float64)
    sc = np.tanh(fin @ v + c)
    sc = np.where(att_mask, -1e4, sc)
    sc = sc - sc.max()
    a = np.exp(sc)
    a = a / a.sum()
    mix = a @ fin
    ms = np.concatenate([span, mix])
    lg = ms @ tag_w.astype(np.float64).T + tag_b.astype(np.float64)
    lg = lg - lg.max()
    return (lg - np.log(np.exp(lg).sum())).astype(np.float32)


def kernel(en_output, lengths, frag_b, frag_s, frag_e, att_w, att_b, tag_w,
           tag_b):
    global LAST_RESULT
    en_output = np.asarray(en_output, dtype=np.float32)
    lengths = np.asarray(lengths).astype(np.int64)
    frag_b = np.asarray(frag_b).astype(np.int64)
    frag_s = np.asarray(frag_s).astype(np.int64)
    frag_e = np.asarray(frag_e).astype(np.int64)
    att_w = np.asarray(att_w, dtype=np.float32)
    att_b = np.asarray(att_b, dtype=np.float32)
    tag_w = np.asarray(tag_w, dtype=np.float32)
    tag_b = np.asarray(tag_b, dtype=np.float32)

    seq_len = float(lengths[0])
    if seq_len not in _compiled:
        _compiled[seq_len] = _build(seq_len)
    nc = _compiled[seq_len]

    in_maps, assign, overflow = _host_prep(
        en_output, lengths, frag_b, frag_s, frag_e, att_w, att_b, tag_w, tag_b)

    res = run_bass_kernel_spmd(nc, in_maps, core_ids=list(range(NCORES)),
                               trace=TRACE)
    LAST_RESULT = res

    out = np.empty((F, T), dtype=np.float32)
    per_core = [res.results[i]["out"].reshape(C, T) for i in range(NCORES)]
    cores = assign[:, 0]
    slots = assign[:, 1]
    for core in range(NCORES):
        sel = cores == core
        out[sel] = per_core[core][slots[sel]]
    for i in overflow:
        out[i] = _host_fragment(en_output, lengths, int(frag_s[i]),
                                int(frag_e[i]), int(frag_b[i]), att_w, att_b,
                                tag_w, tag_b, seq_len)
    return out


# revision 26
# speedup vs baseline: 1.4761x; 1.4761x over previous
"""Trainium2 Bass kernel for nn_AspectModel (span-attention aspect tagger).

Strategy: batch-shard the 32 sentences 4-per-core across 8 NeuronCores; route
each fragment (host-side) to the core owning its sentence, padded to 64 slots
per sentence (256 slots/core).  All heavy math runs on-chip:
  - span features (l_word / word_state / r_word) via a masks-matmul against
    the sentence hidden states (one-hot + in-span masks built on-chip),
  - v = span @ att_w and c = span @ att_b as dense matmuls over all slots,
  - attention scores via a PE matmul of V against the transposed memory
    (transpose done by the DMA xbar in bf16),
  - masked softmax (fused exp+sum) and mix via a second masks-matmul,
  - tag logits + log_softmax.
Matmul operands are cast to bf16 on-chip (f32 PSUM accumulation); the f32
tensor-engine path costs 2 passes per matmul, bf16 costs 1.
Each core returns its own [256, 5] slot outputs; the host scatters them back
into the full [1024, 5] output.  No collectives needed.
"""

import sys
import types

import numpy as np

# Optional shim so run_bass_kernel_spmd(trace=True) works in containers where
# antenv.axon_hooks is missing (profiling only; correctness path unaffected).
try:
    import antenv.axon_hooks  # noqa: F401
except ImportError:
    try:
        from trn_agent_boot.trn_boot import _ntff_profile_via_ctypes

        _hook = _ntff_profile_via_ctypes("/opt/axon/libaxon_pjrt.so")
        _mod = types.ModuleType("antenv.axon_hooks")
        _mod.get_axon_ntff_profile_hook = lambda: _hook
        _mod.set_axon_ntff_profile_hook = lambda h: None
        sys.modules["antenv.axon_hooks"] = _mod
    except Exception:
        pass

import concourse.bass as bass  # noqa: E402
import concourse.tile as tile  # noqa: E402
from concourse import bacc, mybir  # noqa: E402
from concourse import bass_utils  # noqa: E402
from concourse.bass_utils import run_bass_kernel_spmd  # noqa: E402

# No artifact bucket in the sandbox; make tracing's upload step a no-op.
bass_utils.upload_artifacts = lambda tmpdir: f"local:{tmpdir}"

F32 = mybir.dt.float32
BF16 = mybir.dt.bfloat16
I32 = mybir.dt.int32
ALU = mybir.AluOpType
ACT = mybir.ActivationFunctionType

B, S, D, F, T = 32, 256, 512, 1024, 5
NCORES = 8
SEN = 4          # sentences per core
G = 64           # fragment slots per sentence
C = SEN * G      # 256 fragment slots per core
D3 = 3 * D

TRACE = False
LAST_RESULT = None  # BassKernelResults of the most recent run (for test.py)

# Build-time knobs (for A/B experiments; defaults are the shipped config).
OPTS = {
    "memt_eng": "sync",     # engine issuing the memT xbar transposes
    "tiny_eng": "sync",     # engine issuing the small input DMAs
    "mk_first": True,       # emit masksT DVE ops before the bf16 casts
    "ln_dep": True,         # order Ln after both Exp (ACT table thrash)
}

_compiled = {}


def _build(seq_len: float):
    """Build + compile the per-core SPMD graph (identical on all 8 cores)."""
    nc = bacc.Bacc("TRN2", target_bir_lowering=False, debug=False,
                   num_devices=NCORES)

    # All inputs are laid out host-side as [partition, free] so every DMA is a
    # dense per-partition contiguous read.
    x_d = nc.dram_tensor("x", [128, 2 * SEN, D], F32, kind="ExternalInput")
    aw_d = nc.dram_tensor("aw", [128, 12, D], F32, kind="ExternalInput")
    ab_d = nc.dram_tensor("ab", [128, 12], F32, kind="ExternalInput")
    tw_d = nc.dram_tensor("tw", [128, 16, T], F32, kind="ExternalInput")
    tb_d = nc.dram_tensor("tb", [1, T], F32, kind="ExternalInput")
    fs_r_d = nc.dram_tensor("fs_r", [1, C], F32, kind="ExternalInput")
    fm_r_d = nc.dram_tensor("fm_r", [1, C], F32, kind="ExternalInput")
    fs_c_d = nc.dram_tensor("fs_c", [128, 2], F32, kind="ExternalInput")
    fm_c_d = nc.dram_tensor("fm_c", [128, 2], F32, kind="ExternalInput")
    ln_c_d = nc.dram_tensor("ln_c", [128, 2], F32, kind="ExternalInput")
    out_d = nc.dram_tensor("out", [2, 128, T], F32, kind="ExternalOutput")

    with tile.TileContext(nc) as tc:
        with (
            tc.tile_pool(name="persist", bufs=1) as pp,
            tc.tile_pool(name="work", bufs=2) as wp,
            tc.tile_pool(name="psum", bufs=2, space="PSUM") as psp,
        ):
            # ---- persistent SBUF tensors ----
            x_sb = pp.tile([128, 2 * SEN, D], F32, tag="x_sb")
            x_bf = pp.tile([128, 2 * SEN, D], BF16, tag="x_bf")
            aw_sb = pp.tile([128, 12, D], F32, tag="aw_sb")
            aw_bf = pp.tile([128, 12, D], BF16, tag="aw_bf")
            ab_sb = pp.tile([128, 12], F32, tag="ab_sb")
            ab_bf = pp.tile([128, 12], BF16, tag="ab_bf")
            tw_sb = pp.tile([128, 16, T], F32, tag="tw_sb")
            tw_bf = pp.tile([128, 16, T], BF16, tag="tw_bf")
            tb_sb = pp.tile([1, T], F32, tag="tb_sb")
            tbb = pp.tile([128, T], F32, tag="tbb")
            fs_r = pp.tile([1, C], F32, tag="fs_r")
            fm_r = pp.tile([1, C], F32, tag="fm_r")
            fs_c = pp.tile([128, 2], F32, tag="fs_c")
            fm_c = pp.tile([128, 2], F32, tag="fm_c")
            ln_c = pp.tile([128, 2], F32, tag="ln_c")
            iota_i = pp.tile([128, S], I32, tag="iota_i")
            iota_f = pp.tile([128, S], F32, tag="iota_f")
            iota_n = pp.tile([128, S], F32, tag="iota_n")
            fs_b = pp.tile([128, C], F32, tag="fs_b")
            fm_b = pp.tile([128, C], F32, tag="fm_b")
            mkT = [pp.tile([128, 3, C], BF16, tag=f"mkT{k}", name=f"mkT{k}")
                   for k in range(2)]
            spanT = pp.tile([128, 12, C], BF16, tag="spanT")
            v_sb = pp.tile([128, 4, C], BF16, tag="v_sb")
            c_sb = pp.tile([128, 2], F32, tag="c_sb")
            memT = pp.tile([128, SEN, 2, 4, 128], BF16, tag="memT")
            mixT = pp.tile([128, 4, C], BF16, tag="mixT")

            tiny = nc.sync if OPTS["tiny_eng"] == "sync" else nc.scalar
            memt_eng = nc.sync if OPTS["memt_eng"] == "sync" else nc.scalar

            # ---- input DMAs (x and aw, the big loads, go first on sync) ----
            tiny.dma_start(fs_r[:], fs_r_d.ap())
            tiny.dma_start(fm_r[:], fm_r_d.ap())
            tiny.dma_start(fs_c[:], fs_c_d.ap())
            tiny.dma_start(fm_c[:], fm_c_d.ap())
            tiny.dma_start(ln_c[:], ln_c_d.ap())
            nc.scalar.dma_start(tb_sb[:], tb_d.ap())
            nc.scalar.dma_start(ab_sb[:], ab_d.ap())
            nc.scalar.dma_start(tw_sb[:], tw_d.ap())
            for half in range(2):
                nc.sync.dma_start(x_sb[:, half * 4:(half + 1) * 4, :],
                                  x_d.ap()[:, half * 4:(half + 1) * 4, :])
            for third in range(3):
                nc.sync.dma_start(aw_sb[:, third * 4:(third + 1) * 4, :],
                                  aw_d.ap()[:, third * 4:(third + 1) * 4, :])

            # ---- constants ----
            neg4 = pp.tile([128, 1], F32, tag="neg4")
            nc.gpsimd.memset(neg4[:], -1.0e4)
            nc.gpsimd.iota(iota_i[:], pattern=[[1, S]], channel_multiplier=0)
            nc.vector.tensor_copy(iota_f[:], iota_i[:])
            nc.vector.tensor_scalar_mul(iota_n[:], iota_f[:], -1.0)
            nc.gpsimd.partition_broadcast(fs_b[:], fs_r[:])
            nc.gpsimd.partition_broadcast(fm_b[:], fm_r[:])
            nc.gpsimd.partition_broadcast(tbb[:], tb_sb[:])

            def emit_masks():
                # masksT in [s, slot] layout (rhs of the span masks-matmul):
                # component 0: one-hot at frag_s, 1: in-span, 2: at fe-1.
                # partition-index iota per s-chunk: value = k*128 + p
                piota_i = pp.tile([128, 2], I32, tag="piota_i",
                                  name="piota_i")
                piota = pp.tile([128, 2], F32, tag="piota", name="piota")
                nc.gpsimd.iota(piota_i[:], pattern=[[128, 2]],
                               channel_multiplier=1)
                nc.vector.tensor_copy(piota[:], piota_i[:])
                for k in range(2):
                    ip = piota[:, k:k + 1]  # [128,1] = s index in chunk k
                    ge = wp.tile([128, C], F32, tag="msk_ge", name="msk_ge")
                    le = wp.tile([128, C], F32, tag="msk_le", name="msk_le")
                    # fs <= s  and  fe-1 >= s
                    nc.vector.tensor_scalar(ge[:], fs_b[:], ip, None,
                                            op0=ALU.is_le)
                    nc.vector.tensor_scalar(le[:], fm_b[:], ip, None,
                                            op0=ALU.is_ge)
                    nc.vector.tensor_scalar(mkT[k][:, 0, :], fs_b[:], ip,
                                            None, op0=ALU.is_equal)
                    nc.vector.tensor_tensor(mkT[k][:, 1, :], ge[:], le[:],
                                            op=ALU.mult)
                    nc.vector.tensor_scalar(mkT[k][:, 2, :], fm_b[:], ip,
                                            None, op0=ALU.is_equal)

            def emit_casts():
                for c8 in range(2 * SEN):
                    nc.vector.tensor_copy(x_bf[:, c8, :], x_sb[:, c8, :])
                for kk in range(12):
                    nc.any.tensor_copy(aw_bf[:, kk, :], aw_sb[:, kk, :])
                nc.any.tensor_copy(ab_bf[:], ab_sb[:])
                nc.any.tensor_copy(tw_bf[:], tw_sb[:])

            if OPTS["mk_first"]:
                emit_masks()
                emit_casts()
            else:
                emit_casts()
                emit_masks()

            # ---- memT[d, s] via DMA xbar transpose (bf16) ----
            # One big transpose per 2-sentence half: [128, 2048] -> logical
            # [2048, 128]; row r = (l*2+k)*512 + d lands at chunk
            # m = (l*2+k)*4 + dj, partition d%128.  memT is [128, l, k, dj, s'].
            for hf in range(2):
                memt_eng.dma_start_transpose(
                    memT[:, 2 * hf:2 * hf + 2, :, :, :],
                    x_bf[:, hf * 4:(hf + 1) * 4, :])

            # ---- span masks-matmul: spanT[3D, C] (l_word | word_state | r_word)
            sc_span = nc.named_scope("spanmm"); sc_span.__enter__()
            for l in range(SEN):
                for j0 in range(2):  # pairs of D chunks of the word vectors
                    ps = psp.tile([128, 2, 3, G], F32, tag="psm")
                    for dj in range(2):
                        j = j0 * 2 + dj
                        for k in range(2):
                            nc.tensor.matmul(
                                ps[:, dj, :, :],
                                x_bf[:, l * 2 + k, j * 128:(j + 1) * 128],
                                mkT[k][:, :, l * G:(l + 1) * G],
                                start=(k == 0), stop=(k == 1),
                            )
                    nc.any.tensor_copy(
                        spanT[:, j0 * 6:j0 * 6 + 6, l * G:(l + 1) * G], ps[:])

            sc_span.__exit__(None, None, None)
            # ---- v = span @ att_w  (stored transposed: V[d, slot])
            sc_v = nc.named_scope("vmm"); sc_v.__enter__()
            for m0 in range(2):
                pv = psp.tile([128, 2, C], F32, tag="p2k")
                for mj in range(2):
                    m = m0 * 2 + mj
                    for kk in range(12):
                        nc.tensor.matmul(
                            pv[:, mj, :],
                            aw_bf[:, kk, m * 128:(m + 1) * 128],
                            spanT[:, kk, :],
                            start=(kk == 0), stop=(kk == 11),
                        )
                nc.any.tensor_copy(v_sb[:, m0 * 2:m0 * 2 + 2, :], pv[:])

            sc_v.__exit__(None, None, None)
            # ---- c = span @ att_b  ([slot, 1] per 128-slot chunk)
            sc_c = nc.named_scope("cmm"); sc_c.__enter__()
            for q in range(2):
                pc = psp.tile([128, 1], F32, tag="pout")
                for kk in range(12):
                    nc.tensor.matmul(
                        pc[:],
                        spanT[:, kk, q * 128:(q + 1) * 128],
                        ab_bf[:, kk:kk + 1],
                        start=(kk == 0), stop=(kk == 11),
                    )
                nc.any.tensor_copy(c_sb[:, q:q + 1], pc[:])

            sc_c.__exit__(None, None, None)
            # ---- per sentence-pair attention + mix + logits ----
            sh_t = pp.tile([128, 2, T], F32, tag="sh_t")
            se_t = pp.tile([128, 2], F32, tag="se_t")
            ex2_insts = []
            for q in range(2):
                gp = psp.tile([128, S], F32, tag="p2k")
                for h in range(2):
                    l = 2 * q + h
                    for dj in range(4):
                        nc.tensor.matmul(
                            gp[h * G:(h + 1) * G, :],
                            v_sb[:, dj, l * G:(l + 1) * G],
                            memT[:, l, :, dj, :],
                            start=(dj == 0), stop=(dj == 3),
                        )

                fs_q = fs_c[:, q:q + 1]
                fm_q = fm_c[:, q:q + 1]
                ln_q = ln_c[:, q:q + 1]
                c_q = c_sb[:, q:q + 1]

                t1 = wp.tile([128, S], F32, tag="t1")
                t2 = wp.tile([128, S], F32, tag="t2")
                dm = wp.tile([128, S], F32, tag="dm")
                pwr = wp.tile([128, S], F32, tag="pwr")
                noti = wp.tile([128, S], F32, tag="noti")
                klen = wp.tile([128, S], F32, tag="klen")
                kp = wp.tile([128, S], F32, tag="kp")
                pw = wp.tile([128, S], F32, tag="pw")
                sg = wp.tile([128, S], F32, tag="sg")
                th = wp.tile([128, S], F32, tag="th")
                thm = wp.tile([128, S], F32, tag="thm")
                ex = wp.tile([128, S], F32, tag="ex")
                wf = wp.tile([128, S], BF16, tag="wf")
                den = wp.tile([128, 1], F32, tag="den")
                rden = wp.tile([128, 1], F32, tag="rden")

                # pos-distance / pos-weight / masks (slot-major [slot, s])
                nc.vector.tensor_scalar(t1[:], iota_n[:], fs_q, None,
                                        op0=ALU.add)            # fs - s
                nc.vector.tensor_scalar(t2[:], iota_f[:], fm_q, None,
                                        op0=ALU.subtract)       # s - (fe-1)
                nc.vector.tensor_tensor(dm[:], t1[:], t2[:], op=ALU.max)
                nc.vector.tensor_scalar(pwr[:], dm[:], -1.0 / seq_len, 1.0,
                                        op0=ALU.mult, op1=ALU.add)
                nc.vector.tensor_single_scalar(noti[:], dm[:], 0.0,
                                               op=ALU.is_gt)    # not in span
                nc.vector.tensor_scalar(klen[:], iota_f[:], ln_q, None,
                                        op0=ALU.is_lt)          # s < len
                nc.vector.tensor_tensor(kp[:], noti[:], klen[:], op=ALU.mult)
                nc.vector.tensor_tensor(pw[:], pwr[:], noti[:], op=ALU.mult)

                # scores -> tanh -> masked exp (+denominator) -> mix weights
                nc.vector.tensor_tensor(sg[:], pw[:], gp[:], op=ALU.mult)
                nc.scalar.activation(th[:], sg[:], ACT.Tanh, bias=c_q)
                nc.vector.scalar_tensor_tensor(thm[:], th[:], 1.0e4, kp[:],
                                               op0=ALU.add, op1=ALU.mult)
                nc.scalar.activation(ex[:], thm[:], ACT.Exp, bias=neg4[:],
                                     accum_out=den[:])
                nc.vector.reciprocal(rden[:], den[:])
                nc.vector.scalar_tensor_tensor(wf[:], ex[:], rden[:], pw[:],
                                               op0=ALU.mult, op1=ALU.mult)

                # transpose mix weights to [s, slot] via DMA xbar (bf16)
                wT = wp.tile([128, 2, 128], BF16, tag="wT")
                eng = nc.sync if q == 0 else nc.scalar
                eng.dma_start_transpose(wT[:], wf[:])

                # mixT[d, slot] = sum_s mem[s, d] * w[slot, s]
                for h in range(2):
                    l = 2 * q + h
                    pm = psp.tile([128, 4, G], F32, tag="pmix")
                    for dj in range(4):
                        for k in range(2):
                            nc.tensor.matmul(
                                pm[:, dj, :],
                                x_bf[:, l * 2 + k, dj * 128:(dj + 1) * 128],
                                wT[:, k, h * G:(h + 1) * G],
                                start=(k == 0), stop=(k == 1),
                            )
                    nc.any.tensor_copy(mixT[:, :, l * G:(l + 1) * G], pm[:])

                # logits = [span | mix] @ tag_w.T + tag_b
                pl = psp.tile([128, T], F32, tag="pout")
                for kk in range(12):
                    nc.tensor.matmul(
                        pl[:], spanT[:, kk, q * 128:(q + 1) * 128],
                        tw_bf[:, kk, :], start=(kk == 0), stop=False)
                for dj in range(4):
                    nc.tensor.matmul(
                        pl[:], mixT[:, dj, q * 128:(q + 1) * 128],
                        tw_bf[:, 12 + dj, :], start=False, stop=(dj == 3))

                lg = wp.tile([128, T], F32, tag="lg")
                mx = wp.tile([128, 1], F32, tag="mx")
                nc.vector.tensor_tensor(lg[:], pl[:], tbb[:], op=ALU.add)
                nc.vector.tensor_reduce(mx[:], lg[:], axis=mybir.AxisListType.X,
                                        op=ALU.max)
                nc.vector.tensor_scalar(sh_t[:, q, :], lg[:], mx[:], None,
                                        op0=ALU.subtract)
                ex2 = wp.tile([128, T], F32, tag="ex2")
                ex2_insts.append(
                    nc.scalar.activation(ex2[:], sh_t[:, q, :], ACT.Exp,
                                         accum_out=se_t[:, q:q + 1]))

            # ---- log-softmax epilogue (Ln grouped after all Exp/Tanh) ----
            for q in range(2):
                lse = wp.tile([128, 1], F32, tag="lse")
                res = wp.tile([128, T], F32, tag="res")
                ln_inst = nc.scalar.activation(lse[:], se_t[:, q:q + 1],
                                               ACT.Ln)
                if OPTS["ln_dep"]:
                    tile.add_dep_helper(ln_inst.ins, ex2_insts[1].ins,
                                        sync=False,
                                        reason="group Ln after all Exp")
                nc.vector.tensor_scalar(res[:], sh_t[:, q, :], lse[:], None,
                                        op0=ALU.subtract)
                nc.sync.dma_start(out_d.ap()[q], res[:])

    nc.compile()
    return nc


def _host_prep(en_output, lengths, frag_b, frag_s, frag_e, att_w, att_b,
               tag_w, tag_b):
    """Shard + relayout inputs.  Returns (in_maps, assign, overflow)."""
    # replicated weights, permuted so spanT chunk kk = 3*j + comp maps to
    # att rows comp*512 + j*128 : .. + 128.
    perm = np.concatenate([
        np.arange(comp * D + j * 128, comp * D + (j + 1) * 128)
        for j in range(4) for comp in range(3)
    ])
    aw_np = att_w[perm].reshape(12, 128, D).transpose(1, 0, 2).reshape(128, 12, D)
    ab_np = att_b[perm].reshape(12, 128).T.copy()
    tw_rows = np.concatenate([tag_w[:, perm].T,
                              tag_w[:, D3:].T], axis=0)  # [2048, 5]
    tw_np = tw_rows.reshape(16, 128, T).transpose(1, 0, 2).reshape(128, 16, T)
    tb_np = tag_b.reshape(1, T)

    aw_np = np.ascontiguousarray(aw_np, dtype=np.float32)
    ab_np = np.ascontiguousarray(ab_np, dtype=np.float32)
    tw_np = np.ascontiguousarray(tw_np, dtype=np.float32)
    tb_np = np.ascontiguousarray(tb_np, dtype=np.float32)

    assign = np.full((F, 2), -1, dtype=np.int64)  # (core, slot) per fragment
    counts = np.zeros((NCORES, SEN), dtype=np.int64)
    overflow = []
    in_maps = []

    fs_slot = np.zeros((NCORES, C), np.float32)
    fm_slot = np.zeros((NCORES, C), np.float32)
    ln_slot = np.full((NCORES, C), float(S), np.float32)

    for i in range(F):
        b = int(frag_b[i])
        core, l = b // SEN, b % SEN
        k = counts[core, l]
        if k >= G:
            overflow.append(i)
            continue
        counts[core, l] += 1
        slot = l * G + k
        assign[i] = (core, slot)
        fs_slot[core, slot] = frag_s[i]
        fm_slot[core, slot] = frag_e[i] - 1
        ln_slot[core, slot] = lengths[b]

    for core in range(NCORES):
        xs = en_output[core * SEN:(core + 1) * SEN]  # [4, 256, 512]
        x_np = np.ascontiguousarray(
            xs.reshape(SEN, 2, 128, D).transpose(2, 0, 1, 3)
              .reshape(128, 2 * SEN, D), dtype=np.float32)
        in_maps.append({
            "x": x_np, "aw": aw_np, "ab": ab_np, "tw": tw_np, "tb": tb_np,
            "fs_r": fs_slot[core].reshape(1, C),
            "fm_r": fm_slot[core].reshape(1, C),
            "fs_c": np.ascontiguousarray(fs_slot[core].reshape(2, 128).T),
            "fm_c": np.ascontiguousarray(fm_slot[core].reshape(2, 128).T),
            "ln_c": np.ascontiguousarray(ln_slot[core].reshape(2, 128).T),
        })
    return in_maps, assign, overflow


def _host_fragment(en_output, lengths, s, e, b, att_w, att_b, tag_w, tag_b,
                   seq_len):
    """Numpy fallback for (vanishingly rare) slot-overflow fragments."""
    mem = en_output[b].astype(np.float64)
    ws = mem[s:e].sum(0)
    span = np.concatenate([mem[s], ws, mem[e - 1]])
    pos = np.arange(S)
    in_span = (pos >= s) & (pos < e)
    att_mask = in_span | (pos >= lengths[b])
    dis = np.where(pos < s, s - pos,
                   np.where(pos >= e, pos - e + 1, seq_len)).astype(np.float64)
    pwv = 1.0 - dis / seq_len
    fin = pwv[:, None] * mem
    v = span @ att_w.astype(np.float64)
    c = span @ att_b.astype(np.float64)
    sc = np.tanh(fin @ v + c)
    sc = np.where(att_mask, -1e4, sc)
    sc = sc - sc.max()
    a = np.exp(sc)
    a = a / a.sum()
    mix = a @ fin
    ms = np.concatenate([span, mix])
    lg = ms @ tag_w.astype(np.float64).T + tag_b.astype(np.float64)
    lg = lg - lg.max()
    return (lg - np.log(np.exp(lg).sum())).astype(np.float32)


def kernel(en_output, lengths, frag_b, frag_s, frag_e, att_w, att_b, tag_w,
           tag_b):
    global LAST_RESULT
    en_output = np.asarray(en_output, dtype=np.float32)
    lengths = np.asarray(lengths).astype(np.int64)
    frag_b = np.asarray(frag_b).astype(np.int64)
    frag_s = np.asarray(frag_s).astype(np.int64)
    frag_e = np.asarray(frag_e).astype(np.int64)
    att_w = np.asarray(att_w, dtype=np.float32)
    att_b = np.asarray(att_b, dtype=np.float32)
    tag_w = np.asarray(tag_w, dtype=np.float32)
    tag_b = np.asarray(tag_b, dtype=np.float32)

    seq_len = float(lengths[0])
    key = (seq_len, tuple(sorted(OPTS.items())))
    if key not in _compiled:
        _compiled[key] = _build(seq_len)
    nc = _compiled[key]

    in_maps, assign, overflow = _host_prep(
        en_output, lengths, frag_b, frag_s, frag_e, att_w, att_b, tag_w, tag_b)

    res = run_bass_kernel_spmd(nc, in_maps, core_ids=list(range(NCORES)),
                               trace=TRACE)
    LAST_RESULT = res

    out = np.empty((F, T), dtype=np.float32)
    per_core = [res.results[i]["out"].reshape(C, T) for i in range(NCORES)]
    cores = assign[:, 0]
    slots = assign[:, 1]
    for core in range(NCORES):
        sel = cores == core
        out[sel] = per_core[core][slots[sel]]
    for i in overflow:
        out[i] = _host_fragment(en_output, lengths, int(frag_s[i]),
                                int(frag_e[i]), int(frag_b[i]), att_w, att_b,
                                tag_w, tag_b, seq_len)
    return out
